# revision 3
# baseline (speedup 1.0000x reference)
"""Conditional contrastive loss on 8 TRN2 NeuronCores (Bass/Tile).

Strategy (data-parallel over rows, per sharding hint):
  - Each core owns 512 rows (of 4096) of inst_embed ("x") and proxy ("p").
  - Host passes transposed layouts (matmul-native [D, n]) - no device transposes.
  - Device normalizes embeddings (squares on GPSIMD, partition-reduction via an
    all-ones matmul whose M=128 output doubles as a partition-broadcast,
    ACT sqrt + DVE reciprocal, DVE column-scale into fp32r).
  - Similarity rows sim[i, j] for the core's i-block are computed as fp32r
    matmuls accumulated in PSUM over 4 K-chunks.
  - The positive-selection mask is rank-C:  mask = onehot(labels) @ negative_mask.
    One extra bf16 matmul (exact: entries are 0 or BIG) accumulated into the
    same PSUM bank turns denominator logits into numerator logits; the exp
    bias subtracts BIG so unmasked entries underflow to 0.
  - exp() runs on the scalar engine with accum_out, which yields the row sums
    (den and num) for free - no vector-engine work in the main loop.
  - Device emits ln(den), ln(num) per row for both matrices ([512, 4] f32 per
    core); the host does the final O(N) mean + gather across cores.
"""
import numpy as np
import ml_dtypes

import concourse.bacc as bacc
import concourse.tile as tile
from concourse import mybir, bass_utils

N_FULL = 4096
D = 512
C = 100
N_CORES = 8
RP = N_FULL // N_CORES  # rows per core = 512
P = 128                 # SBUF partitions
KC = D // P             # contraction chunks = 4
JT = 512                # columns per PSUM bank
JG = 2048               # columns per PSUM group (4 banks)
NG = N_FULL // JG       # groups per (i-tile, matrix) = 2
IT = RP // P            # i-tiles per core = 4

F32 = mybir.dt.float32
F32R = mybir.dt.float32r
BF16 = mybir.dt.bfloat16
AF = mybir.ActivationFunctionType
ALU = mybir.AluOpType

_CACHE = {}


def _build(inv_t: float, bias_den: float, bias_num: float):
    nc = bacc.Bacc("TRN2", target_bir_lowering=False, debug=False,
                   num_devices=N_CORES)

    xT = nc.dram_tensor("xT", [D, N_FULL], F32, kind="ExternalInput")
    xTc = nc.dram_tensor("xTc", [D, RP], F32, kind="ExternalInput")
    pTc = nc.dram_tensor("pTc", [D, RP], F32, kind="ExternalInput")
    nm = nc.dram_tensor("nm", [C, N_FULL], BF16, kind="ExternalInput")
    oh = nc.dram_tensor("oh", [C, RP], BF16, kind="ExternalInput")
    out = nc.dram_tensor("out", [RP, 4], F32, kind="ExternalOutput")

    with tile.TileContext(nc) as tc:
        with (
            tc.tile_pool(name="xpool", bufs=5) as xpool,
            tc.tile_pool(name="big", bufs=1) as big,
            tc.tile_pool(name="scratch", bufs=3) as scratch,
            tc.tile_pool(name="lhs", bufs=2) as lhs,
            tc.tile_pool(name="small", bufs=1) as small,
            tc.tile_pool(name="ps", bufs=2, space="PSUM") as pspool,
        ):
            # ---- constants ----
            ones_f = small.tile([P, P], F32, name="ones_f")
            nc.vector.memset(ones_f[:], 1.0)
            ones_r = small.tile([P, P], F32R, name="ones_r")
            nc.vector.tensor_copy(ones_r[:], ones_f[:])
            bias_den_t = small.tile([P, 1], F32, name="bias_den_t")
            nc.vector.memset(bias_den_t[:], bias_den)
            bias_num_t = small.tile([P, 1], F32, name="bias_num_t")
            nc.vector.memset(bias_num_t[:], bias_num)

            # ---- loads ----
            xt = []
            for k in range(KC):
                t = xpool.tile([P, N_FULL], F32, name=f"xt{k}", tag="x")
                nc.sync.dma_start(t[:], xT.ap()[k * P:(k + 1) * P, :])
                xt.append(t)
            xtc = []
            ptc = []
            for k in range(KC):
                t = lhs.tile([P, RP], F32, name=f"xtc{k}", tag=f"xtc{k}")
                nc.sync.dma_start(t[:], xTc.ap()[k * P:(k + 1) * P, :])
                xtc.append(t)
                t = lhs.tile([P, RP], F32, name=f"ptc{k}", tag=f"ptc{k}")
                nc.sync.dma_start(t[:], pTc.ap()[k * P:(k + 1) * P, :])
                ptc.append(t)
            nm_t = small.tile([C, N_FULL], BF16, name="nm_t")
            nc.sync.dma_start(nm_t[:], nm.ap())
            oh_t = small.tile([C, RP], BF16, name="oh_t")
            nc.sync.dma_start(oh_t[:], oh.ap())

            # ---- norms of full x (columns of xT) ----
            # squares on gpsimd (idle engine), rounded to f32r
            ps_norm = [
                pspool.tile([P, JG], F32, name=f"ps_norm{g}", tag="ps")
                for g in range(NG)
            ]
            for k in range(KC):
                sq = big.tile([P, N_FULL], F32R, name=f"sq{k}", tag="bigbuf")
                nc.gpsimd.tensor_tensor(sq[:], xt[k][:], xt[k][:], ALU.mult)
                for g in range(NG):
                    for jl in range(JG // JT):
                        j0 = g * JG + jl * JT
                        nc.tensor.matmul(
                            ps_norm[g][:, jl * JT:(jl + 1) * JT],
                            ones_r[:],
                            sq[:, j0:j0 + JT],
                            start=(k == 0), stop=(k == KC - 1),
                        )
            # 1/||x_j|| broadcast over partitions: sqrt (ACT) then recip (DVE)
            b_inv = big.tile([P, N_FULL], F32, name="b_inv", tag="bigbuf")
            for g in range(NG):
                sb = scratch.tile([P, JG], F32, name=f"sb{g}", tag="scr")
                nc.scalar.activation(sb[:], ps_norm[g][:], AF.Sqrt)
                nc.vector.reciprocal(b_inv[:, g * JG:(g + 1) * JG], sb[:])

            # normalized xT in fp32r (rhs for all gram matmuls)
            xn = []
            for k in range(KC):
                t = xpool.tile([P, N_FULL], F32R, name=f"xn{k}", tag="x")
                nc.vector.tensor_tensor(t[:], xt[k][:], b_inv[:], ALU.mult)
                xn.append(t)

            # ---- norms of the core's own x rows and proxy rows ----
            def chunk_norm_inv(src_tiles, label):
                ps_c = pspool.tile([P, JG], F32, name=f"psc_{label}", tag="ps")
                for k in range(KC):
                    sqc = small.tile([P, RP], F32R, name=f"sqc_{label}{k}",
                                     tag="sqc")
                    nc.gpsimd.tensor_tensor(sqc[:], src_tiles[k][:],
                                            src_tiles[k][:], ALU.mult)
                    nc.tensor.matmul(ps_c[:, :RP], ones_r[:], sqc[:],
                                     start=(k == 0), stop=(k == KC - 1))
                sbc = scratch.tile([P, RP], F32, name=f"sbc_{label}", tag="scr")
                nc.scalar.activation(sbc[:], ps_c[:, :RP], AF.Sqrt)
                inv = small.tile([P, RP], F32, name=f"inv_{label}")
                nc.vector.reciprocal(inv[:], sbc[:])
                return inv

            bx_inv = chunk_norm_inv(xtc, "x")
            bp_inv = chunk_norm_inv(ptc, "p")

            # normalized lhsT chunks in fp32r
            xnc = []
            pnc = []
            for k in range(KC):
                t = lhs.tile([P, RP], F32R, name=f"xnc{k}", tag=f"xtc{k}")
                nc.vector.tensor_tensor(t[:], xtc[k][:], bx_inv[:], ALU.mult)
                xnc.append(t)
                t = lhs.tile([P, RP], F32R, name=f"pnc{k}", tag=f"ptc{k}")
                nc.vector.tensor_tensor(t[:], ptc[k][:], bp_inv[:], ALU.mult)
                pnc.append(t)

            # ---- main loop ----
            acc_den = {}
            acc_num = {}
            for it in range(IT):
                for mat in range(2):
                    acc_den[it, mat] = small.tile([P, NG], F32,
                                                  name=f"accd{it}_{mat}")
                    acc_num[it, mat] = small.tile([P, NG], F32,
                                                  name=f"accn{it}_{mat}")

            for it in range(IT):
                i0 = it * P
                for mat in range(2):
                    lh = pnc if mat == 0 else xnc
                    for g in range(NG):
                        ps = pspool.tile([P, JG], F32,
                                         name=f"ps_{it}_{mat}_{g}", tag="ps")
                        for k in range(KC):
                            for jl in range(JG // JT):
                                j0 = g * JG + jl * JT
                                nc.tensor.matmul(
                                    ps[:, jl * JT:(jl + 1) * JT],
                                    lh[k][:, i0:i0 + P],
                                    xn[k][:, j0:j0 + JT],
                                    start=(k == 0), stop=False,
                                )
                        z = scratch.tile([P, JG], BF16,
                                         name=f"z_{it}_{mat}_{g}", tag="scr")
                        nc.scalar.activation(
                            z[:], ps[:], AF.Exp,
                            bias=bias_den_t[:], scale=inv_t,
                            accum_out=acc_den[it, mat][:, g:g + 1],
                        )
                        for jl in range(JG // JT):
                            j0 = g * JG + jl * JT
                            nc.tensor.matmul(
                                ps[:, jl * JT:(jl + 1) * JT],
                                oh_t[:, i0:i0 + P],
                                nm_t[:, j0:j0 + JT],
                                start=False, stop=True,
                            )
                        z2 = scratch.tile([P, JG], BF16,
                                          name=f"z2_{it}_{mat}_{g}", tag="scr")
                        nc.scalar.activation(
                            z2[:], ps[:], AF.Exp,
                            bias=bias_num_t[:], scale=inv_t,
                            accum_out=acc_num[it, mat][:, g:g + 1],
                        )

            # ---- tail: sum group partials, take logs, store ----
            sums = small.tile([P, 4 * IT], F32, name="sums")
            lns = small.tile([P, 4 * IT], F32, name="lns")
            for it in range(IT):
                for mat in range(2):
                    cd = it * 4 + mat * 2
                    nc.vector.tensor_reduce(sums[:, cd:cd + 1],
                                            acc_den[it, mat][:],
                                            mybir.AxisListType.X, ALU.add)
                    nc.vector.tensor_reduce(sums[:, cd + 1:cd + 2],
                                            acc_num[it, mat][:],
                                            mybir.AxisListType.X, ALU.add)
            nc.scalar.activation(lns[:], sums[:], AF.Ln)
            for it in range(IT):
                nc.sync.dma_start(out.ap()[it * P:(it + 1) * P, :],
                                  lns[:, it * 4:(it + 1) * 4])

    nc.compile()
    return nc


def kernel(inst_embed, proxy, negative_mask, labels, temperature, margin):
    t = float(np.asarray(temperature))
    m = float(np.asarray(margin))
    # Mask penalty: exact in bf16; BIG/t ~ 40 keeps masked exp args within
    # the ACT exp table range while underflowing to exactly 0.
    big = float(np.asarray(40.0 * t, dtype=ml_dtypes.bfloat16))
    inv_t = 1.0 / t
    bias_den = -m / t
    bias_num = -(m + big) / t

    key = (t, m)
    if key not in _CACHE:
        _CACHE[key] = _build(inv_t, bias_den, bias_num)
    nc = _CACHE[key]

    x = np.asarray(inst_embed, dtype=np.float32)
    p = np.asarray(proxy, dtype=np.float32)
    nm16 = np.asarray(negative_mask, dtype=np.float32).astype(ml_dtypes.bfloat16)
    lab = np.asarray(labels).astype(np.int64)

    xT = np.ascontiguousarray(x.T)
    in_maps = []
    for c in range(N_CORES):
        r0 = c * RP
        rows = slice(r0, r0 + RP)
        ohc = np.zeros((C, RP), dtype=ml_dtypes.bfloat16)
        ohc[lab[rows], np.arange(RP)] = ml_dtypes.bfloat16(big)
        in_maps.append({
            "xT": xT,
            "xTc": np.ascontiguousarray(x[rows].T),
            "pTc": np.ascontiguousarray(p[rows].T),
            "nm": nm16,
            "oh": ohc,
        })

    res = bass_utils.run_bass_kernel_spmd(nc, in_maps, core_ids=list(range(N_CORES)))
    outs = np.concatenate([res.results[c]["out"] for c in range(N_CORES)], axis=0)
    ld_p, ln_p, ld_i, ln_i = (outs[:, q].astype(np.float64) for q in range(4))
    loss = (-2.0 * np.log(t)
            + (ld_p - ln_p).mean()
            + (ld_i - ln_i).mean())
    return np.float32(loss)
